# revision 21
# baseline (speedup 1.0000x reference)
"""Trainium2 Bass kernel for the CSMHP (clustered self-exciting Hawkes
process) negative log-likelihood, distributed over 8 NeuronCores.

Math
----
The excitation E[c,i] = sum_{j<i} exp(-beta_c (t_i - t_j)) obeys
E_i = d_i (E_{i-1} + 1) with d_i = exp(-beta_c (t_i - t_{i-1})) -> one DVE
tensor_tensor_scan.  Own events sit in a (16, 256) layout (partition h*8+c,
h = half-block) so the scan runs 256 steps instead of 512; the second half
is then fixed up with  exc = exp(-beta (t_i - t_mid)) * E(t_mid) + S,
E(t_mid) broadcast from the h=0 partitions by a tiny PE shift matmul.
t_prev is t_own shifted one event, so only each half's first-predecessor
column ships from the host; the rest of dt is an aliased-AP subtract.

The scan's initial state (dense sum over all prior events, padded to 3584)
comes from a PE replication matmul: lhsT M_beta spreads the 8x448 prior
groups to 64 partitions scaled by beta_c, one ACT exp with accumulate
reduces it, and a second tiny matmul folds the 8 groups per cluster.  The
same bo-build matmul also broadcasts beta_c and -beta_c*tref to 64
partitions for the exp's scale/bias operands.

Intensity_i = sum_c (pt*alpha)[c,i] E[c,i] + sum_c pt[c,i] base[c,i]
(base = mu + gamma t / T): the base term is matmul-accumulated into the
intensity PSUM bank before the scan finishes, so the post-scan path is one
multiply -> accumulating matmul -> Ln(accumulate).

Sharding: 8 contiguous 512-event blocks, one per core, no collectives;
the host sums the per-core partial scalars (the "all-reduce" of the hint).

Engine/latency notes
--------------------
* Inputs ride three small dma_starts (sync: t-chunk + prolog tensor,
  gpsimd: the rest) -- tiny descriptor counts keep the transfers off
  straggling DMA engines whose late completion descriptor otherwise posts
  the input semaphore microseconds late.
* ACT runs a dummy exp first so the ~1.3us activation-table load overlaps
  the input DMA instead of gating the first real exp.
* Same-engine RAW pairs are separated by semaphore self-waits (the
  engine pipelines are not interlocked); every cross-engine edge has an
  explicit semaphore.  CoreSim (8 cores) validates the graph race-free.
* No trailing semaphore cleanup: the runtime end-of-NEFF walk resets all
  semaphores S[3..255] between executions anyway (that walk, ~56
  instructions per engine, is the fixed ~6.6us tail after the program).
"""

import numpy as np

import concourse.bass as bass
from concourse import mybir
from concourse.bass_utils import run_bass_kernel_spmd

F32 = mybir.dt.float32
F32R = mybir.dt.float32r
ALU = mybir.AluOpType
ACT = mybir.ActivationFunctionType

N = 4096
C = 8
NCORES = 8
CHUNK = N // NCORES          # 512 events per core
HB = CHUNK // 2              # 256 per half-block
P16 = 16
PRIOR_PAD = 3584
G = 8
PCOL = PRIOR_PAD // G        # 448
T_WINDOW = 100.0
BIG = 1.0e9

# inA (f32, 16 partitions) column layout; chunk1 = cols [0:512] (sync DMA),
# chunk2 = cols [512:] (gpsimd DMA) so the decay chain starts on chunk1
A_TOWN = 0                   # t16 [16, 256]
A_TP0 = A_TOWN + HB          # [16, 1] t_prev of each half-block's first event
A_PT = A_TP0 + 1
A_SCAL = A_PT + HB           # beta, alpha, mu, gamma/T (tiled x2)
A_ZERO = A_SCAL + 4
A_NBETA = A_ZERO + 1         # -beta16
A_BTMID = A_NBETA + 1        # beta16 * t_mid
A_SHIFT = A_BTMID + 1        # [16, 16] einit shift lhsT
A_I16 = A_SHIFT + 16         # [16, 16] identity (einit union lhsT)
A_MC = A_I16 + 16            # [8, 64] cluster indicator (rows 0:8)
A_I8E = A_MC + 64            # [8, 18] identity|zeros|beta|-beta*tref (rows 0:8)
A_COLS = A_I8E + 18          # 628
A_SPLIT = HB + 1             # 257

# inP (f32r, 16 partitions; prolog data on rows 0:8) column layout
P_MBETA = 0                  # [8, 64]
P_PRI = P_MBETA + 64         # [8, 448]
P_HMASK = P_PRI + PCOL       # [16, 2] half-mask (intensity matmul lhsT)
P_COLS = P_HMASK + 2         # 514

_NC_CACHE = None


def _build_nc(with_dummy: bool = True, redundant: bool = False):
    """redundant=True double-issues each input DMA from a second engine so
    the waiters release on the first copy -- it measured fastest (17.3us max)
    but showed intermittent low-order result corruption on hardware, so it
    stays off by default."""
    nc = bass.Bass("TRN2", target_bir_lowering=False, debug=False)

    ina_d = nc.dram_tensor("inA", [P16, A_COLS], F32, kind="ExternalInput")
    inp_d = nc.dram_tensor("inP", [P16, P_COLS], F32R, kind="ExternalInput")
    # out cols: 0 = probability column sums (sum rows h*8+c over h);
    # 1 = last-event excitation (rows 8:16); 2 = ll halves (rows 0:2)
    out_d = nc.dram_tensor("out", [P16, 4], F32, kind="ExternalOutput")

    from contextlib import ExitStack

    ctx = ExitStack()
    sb = lambda name, shape, dt=F32: ctx.enter_context(
        nc.sbuf_tensor(name, shape, dt)
    )
    psum = lambda name, shape: ctx.enter_context(
        nc.psum_tensor(name, shape, F32)
    )
    sem = lambda name: ctx.enter_context(nc.semaphore(name))
    with ctx:
        ina = sb("ina", [P16, A_COLS])
        inp = sb("inp", [P16, P_COLS], F32R)
        e64 = sb("e64", [64, PCOL])
        acol64 = sb("acol64", [64, 1])
        bo_sb = sb("bo_sb", [64, 18])
        dt16 = sb("dt16", [P16, HB])
        dec = sb("dec", [P16, HB])
        expg = sb("expg", [P16, HB])
        base = sb("base", [P16, HB])
        pta = sb("pta", [P16, HB])
        exc = sb("exc", [P16, HB])
        excf = sb("excf", [P16, HB])
        qcol = sb("qcol", [P16, 1])
        asb = sb("asb", [P16, 1])
        scratch = sb("scratch", [C, 2])
        pl = sb("pl", [P16, HB], F32R)
        pb = sb("pb", [P16, HB], F32R)
        logi = sb("logi", [2, HB])
        out_stage = sb("out_stage", [P16, 4])
        psum64 = psum("psum64", [64, PCOL])
        bo_ps = psum("bo_ps", [64, 18])
        a_init = psum("a_init", [P16, 1])
        einit = psum("einit", [P16, 1])
        inten = psum("inten", [2, HB])
        s_ina = sem("s_ina")
        s_ina2 = sem("s_ina2")
        s_inp = sem("s_inp")
        s_dve = sem("s_dve")
        s_act = sem("s_act")
        s_pe = sem("s_pe")
        s_stage = sem("s_stage")
        s_v = sem("s_v")
        s_pool = sem("s_pool")
        s_out = sem("s_out")

        ina_ap = ina.ap()
        t16 = ina_ap[:, A_TOWN:A_TOWN + HB]
        tp0_col = ina_ap[:, A_TP0:A_TP0 + 1]
        pt16 = ina_ap[:, A_PT:A_PT + HB]
        beta_col = ina_ap[:, A_SCAL + 0:A_SCAL + 1]
        alpha_col = ina_ap[:, A_SCAL + 1:A_SCAL + 2]
        mu_col = ina_ap[:, A_SCAL + 2:A_SCAL + 3]
        gammat_col = ina_ap[:, A_SCAL + 3:A_SCAL + 4]
        zeros16 = ina_ap[:, A_ZERO:A_ZERO + 1]
        nbeta_col = ina_ap[:, A_NBETA:A_NBETA + 1]
        btmid_col = ina_ap[:, A_BTMID:A_BTMID + 1]
        shift16 = ina_ap[:, A_SHIFT:A_SHIFT + 16]
        i16 = ina_ap[:, A_I16:A_I16 + 16]
        m_c = ina_ap[0:C, A_MC:A_MC + 64]
        i8e = ina_ap[0:C, A_I8E:A_I8E + 18]
        inp_ap = inp.ap()
        m_beta = inp_ap[0:C, P_MBETA:P_MBETA + 64]
        pri_ref = inp_ap[0:C, P_PRI:P_PRI + PCOL]
        hmask = inp_ap[:, P_HMASK:P_HMASK + 2]

        n_prefix = len(nc.m.functions[0].blocks[0].instructions)

        # ---- input DMA issue: inP first (its consumer chain
        # repl->exp->accum->a_init is longer than dt->dec) ----
        nc.sync.dma_start(out=inp.ap(), in_=inp_d.ap()).then_inc(s_inp, 16)
        nc.sync.dma_start(
            out=ina.ap()[:, 0:A_SPLIT], in_=ina_d.ap()[:, 0:A_SPLIT]
        ).then_inc(s_ina, 16)
        nc.gpsimd.dma_start(
            out=ina.ap()[:, A_SPLIT:A_COLS], in_=ina_d.ap()[:, A_SPLIT:A_COLS]
        ).then_inc(s_ina2, 16)
        if redundant:
            nc.gpsimd.dma_start(
                out=ina.ap()[:, 0:A_SPLIT], in_=ina_d.ap()[:, 0:A_SPLIT]
            ).then_inc(s_ina, 16)

        # ---- ACT stream: dummy FIRST so the PWP activation-table load
        # lands before the measured window, then the inP DMA issue ----
        if with_dummy:
            nc.scalar.activation(
                scratch.ap()[:, 0:1], scratch.ap()[:, 1:2], ACT.Exp,
                bias=scratch.ap()[:, 1:2],
            )
        if redundant:
            nc.scalar.dma_start(out=inp.ap(), in_=inp_d.ap()).then_inc(
                s_inp, 16
            )
            nc.scalar.dma_start(
                out=ina.ap()[:, A_SPLIT:A_COLS],
                in_=ina_d.ap()[:, A_SPLIT:A_COLS],
            ).then_inc(s_ina2, 16)
        nc.scalar.wait_ge(s_dve, 2)        # dt16 ready (both sub pieces)
        nc.scalar.wait_ge(s_ina2, 16)      # beta/zeros columns
        nc.scalar.activation(
            dec.ap(), dt16.ap(), ACT.Exp, bias=zeros16, scale=beta_col,
        ).then_inc(s_act, 1)                                       # s_act 1
        nc.scalar.wait_ge(s_pe, 2)         # psum64 (prolog matmul) done
        nc.scalar.wait_ge(s_dve, 3)        # bo_sb (incl. -beta*tref col)
        nc.scalar.activation(
            e64.ap(), psum64.ap()[:, 0:PCOL], ACT.Exp,
            bias=bo_sb.ap()[:, 17:18], accum_out=acol64.ap(),
        ).then_inc(s_act, 1)                                       # s_act 2
        nc.scalar.activation(
            expg.ap(), t16, ACT.Exp, bias=btmid_col, scale=nbeta_col,
        ).then_inc(s_act, 1)                                       # s_act 3
        nc.scalar.wait_ge(s_pe, 5)         # intensity matmul stopped
        # halved Ln so the first accumulator read overlaps the second Ln
        nc.scalar.activation(
            logi.ap()[:, 0:HB // 2], inten.ap()[:, 0:HB // 2], ACT.Ln,
            bias=zeros16[0:2, :], accum_out=out_stage.ap()[0:2, 2:3],
        ).then_inc(s_act, 1)                                       # s_act 4
        nc.scalar.activation(
            logi.ap()[:, HB // 2:HB], inten.ap()[:, HB // 2:HB], ACT.Ln,
            bias=zeros16[0:2, :], accum_out=out_stage.ap()[0:2, 3:4],
        ).then_inc(s_act, 1)                                       # s_act 5
        nc.scalar.wait_ge(s_stage, 2)      # reduce + elast staged
        nc.scalar.wait_ge(s_act, 5)        # drain own Ln accum writes
        nc.scalar.dma_start(out=out_d.ap(), in_=out_stage.ap()).then_inc(
            s_out, 16
        )

        # ---- PE stream ----
        nc.tensor.wait_ge(s_ina2, 16)
        nc.tensor.matmul(
            bo_ps.ap(), m_c, i8e, start=True, stop=True
        ).then_inc(s_pe, 1)                                        # s_pe 1
        nc.tensor.wait_ge(s_inp, 16)
        nc.tensor.matmul(
            psum64.ap(), m_beta, pri_ref, start=True, stop=True
        ).then_inc(s_pe, 1)                                        # s_pe 2
        nc.tensor.wait_ge(s_act, 2)        # acol64 ready
        nc.tensor.wait_ge(s_dve, 3)        # bo_sb copied
        nc.tensor.matmul(
            a_init.ap(), bo_sb.ap()[:, 0:P16], acol64.ap(),
            start=True, stop=True,
        ).then_inc(s_pe, 1)                                        # s_pe 3
        # union bank: A' on the h=0 rows (identity x staged copy) plus the
        # shifted half-0 terminal S_end + f_end*A' on the h=1 rows
        nc.tensor.wait_ge(s_dve, 4)        # asb staged
        nc.tensor.matmul(
            einit.ap(), i16, asb.ap(), start=True, stop=False,
        )
        nc.tensor.wait_ge(s_dve, 5)        # qcol staged
        nc.tensor.matmul(
            einit.ap(), shift16, qcol.ap(), start=False, stop=True,
        ).then_inc(s_pe, 1)                                        # s_pe 4
        nc.tensor.wait_ge(s_pool, 2)
        nc.tensor.matmul(
            inten.ap(), hmask, pb.ap(), start=True, stop=False
        )
        nc.tensor.wait_ge(s_dve, 6)        # pl ready
        nc.tensor.matmul(
            inten.ap(), hmask, pl.ap(), start=False, stop=True
        ).then_inc(s_pe, 1)                                        # s_pe 5
        # ---- DVE stream ----
        nc.vector.wait_ge(s_ina, 16)
        # t_prev is t_own shifted one event right; only each half-block's
        # first event needs the host-supplied predecessor column
        nc.vector.tensor_sub(
            dt16.ap()[:, 1:HB], ina_ap[:, A_TOWN:A_TOWN + HB - 1],
            ina_ap[:, A_TOWN + 1:A_TOWN + HB],
        ).then_inc(s_dve, 1)
        nc.vector.tensor_sub(
            dt16.ap()[:, 0:1], tp0_col, t16[:, 0:1]
        ).then_inc(s_dve, 1)                                       # s_dve 2
        nc.vector.wait_ge(s_pe, 1)
        nc.vector.tensor_copy(bo_sb.ap(), bo_ps.ap()).then_inc(
            s_dve, 1
        )                                                          # s_dve 3
        # zero the ll column so the (rectangular) output DMA reads no
        # uninitialized bytes; precedes the Ln accum via the pl/matmul sems
        nc.vector.memset(out_stage.ap()[:, 2:4], 0.0)
        nc.vector.tensor_scalar(
            out=pta.ap(), in0=pt16, scalar1=alpha_col, scalar2=None,
            op0=ALU.mult,
        )
        nc.vector.wait_ge(s_act, 1)        # dec exp done
        nc.vector.tensor_tensor_scan(
            exc.ap(), dec.ap(), dec.ap(), initial=0.0,
            op0=ALU.mult, op1=ALU.add,
        ).then_inc(s_v, 1)                                         # s_v 1
        nc.vector.wait_ge(s_v, 1)
        nc.vector.wait_ge(s_pe, 3)         # a_init closed
        nc.vector.tensor_copy(asb.ap(), a_init.ap()).then_inc(
            s_dve, 1
        )                                                          # s_dve 4
        nc.vector.wait_ge(s_dve, 4)        # drain own asb write
        nc.vector.wait_ge(s_act, 3)        # expg ready
        nc.vector.scalar_tensor_tensor(
            out=qcol.ap(), in0=expg.ap()[:, HB - 1:HB], scalar=asb.ap(),
            in1=exc.ap()[:, HB - 1:HB], op0=ALU.mult, op1=ALU.add,
        ).then_inc(s_dve, 1)                                       # s_dve 5
        nc.vector.wait_ge(s_pe, 4)         # union bank complete
        nc.vector.scalar_tensor_tensor(
            out=excf.ap(), in0=expg.ap(), scalar=einit.ap(), in1=exc.ap(),
            op0=ALU.mult, op1=ALU.add,
        ).then_inc(s_v, 1)                                         # s_v 2
        nc.vector.wait_ge(s_v, 2)
        nc.vector.tensor_mul(pl.ap(), excf.ap(), pta.ap()).then_inc(
            s_dve, 1
        )                                                          # s_dve 4 (cumulative)
        nc.vector.reduce_sum(
            out_stage.ap()[:, 0:1], pt16, axis=mybir.AxisListType.X
        ).then_inc(s_stage, 1)                                     # s_stage 1
        nc.vector.tensor_copy(
            out_stage.ap()[:, 1:2], excf.ap()[:, HB - 1:HB]
        ).then_inc(s_stage, 1)                                     # s_stage 2
        # ---- Pool: base, pb ----
        nc.gpsimd.wait_ge(s_ina, 16)       # t16 (chunk 1)
        nc.gpsimd.wait_ge(s_ina2, 16)
        nc.gpsimd.tensor_scalar(
            out=base.ap(), in0=t16, scalar1=gammat_col, scalar2=mu_col,
            op0=ALU.mult, op1=ALU.add,
        ).then_inc(s_pool, 1)                                      # s_pool 1
        nc.gpsimd.wait_ge(s_pool, 1)       # drain: Pool is not interlocked
        nc.gpsimd.tensor_mul(pb.ap(), pt16, base.ap()).then_inc(
            s_pool, 1
        )                                                          # s_pool 2

    _strip_entry_scaffolding(nc, n_prefix)
    return nc


def _strip_entry_scaffolding(nc, n_prefix):
    main = nc.m.functions[0].blocks[0]
    drop_types = ("InstMemset", "InstDrain", "InstEventSemaphore")
    kept = [
        inst
        for i, inst in enumerate(main.instructions)
        if i >= n_prefix or type(inst).__name__ not in drop_types
    ]
    main.instructions[:] = kept


def get_nc():
    global _NC_CACHE
    if _NC_CACHE is None:
        _NC_CACHE = _build_nc()
    return _NC_CACHE


def make_in_maps(probability, event_times, mu, gamma, alpha_kernel, beta_kernel):
    t = np.ascontiguousarray(np.asarray(event_times, dtype=np.float32))
    p = np.ascontiguousarray(np.asarray(probability, dtype=np.float32))
    beta = np.asarray(beta_kernel, dtype=np.float32)
    alpha = np.asarray(alpha_kernel, dtype=np.float32)
    mu_ = np.asarray(mu, dtype=np.float32)
    gamma_ = np.asarray(gamma, dtype=np.float32)

    beta16 = np.tile(beta, 2)[:, None]
    scal16 = np.tile(
        np.stack([beta, alpha, mu_, gamma_ / np.float32(T_WINDOW)], axis=1),
        (2, 1),
    )
    zeros16 = np.zeros((P16, 1), np.float32)
    nbeta16 = -beta16
    shift16 = np.zeros((P16, P16), np.float32)
    for c in range(C):
        shift16[c, 8 + c] = 1.0
    m_c = np.zeros((C, 64), np.float32)
    m_beta = np.zeros((C, 64), np.float32)
    for c in range(C):
        for g in range(G):
            m_c[c, c * 8 + g] = 1.0
            m_beta[g, c * 8 + g] = beta[c]
    i8p = np.concatenate(
        [np.eye(C, dtype=np.float32), np.zeros((C, 8), np.float32)], axis=1
    )
    hmask = np.zeros((P16, 2), np.float32)
    for h in (0, 1):
        hmask[h * 8:(h + 1) * 8, h] = 1.0

    in_maps = []
    for k in range(NCORES):
        s = k * CHUNK
        tch = t[s:s + CHUNK]
        tp = np.empty(CHUNK, np.float32)
        if k == 0:
            tp[0] = t[0] - BIG
            tp[1:] = t[:CHUNK - 1]
        else:
            tp[:] = t[s - 1:s + CHUNK - 1]
        ptc = p[s:s + CHUNK, :].T

        t16 = np.stack(
            [np.broadcast_to(tch[h * HB:(h + 1) * HB], (C, HB)) for h in (0, 1)]
        ).reshape(P16, HB)
        tp0 = np.stack(
            [np.full((C, 1), tp[h * HB], np.float32) for h in (0, 1)]
        ).reshape(P16, 1)
        pt16 = np.stack(
            [ptc[:, h * HB:(h + 1) * HB] for h in (0, 1)]
        ).reshape(P16, HB)
        t_mid = np.float32(tch[HB - 1])
        tref_h = np.float32(t[s - 1] if k > 0 else t[0])
        btmid16 = beta16 * np.concatenate(
            [np.full(8, tref_h, np.float32), np.full(8, t_mid, np.float32)]
        )[:, None]

        npri = max(s - 1, 0)
        pri = np.full(PRIOR_PAD, -BIG, np.float32)
        pri[:npri] = t[:npri]
        pri8 = pri.reshape(G, PCOL)
        tref_val = np.float32(t[s - 1] if k > 0 else t[0])

        i8e = np.concatenate(
            [i8p, beta[:, None], -beta[:, None] * tref_val], axis=1,
            dtype=np.float32,
        )
        mci8 = np.vstack(
            [np.concatenate([m_c, i8e], axis=1),
             np.zeros((8, 64 + 18), np.float32)]
        )
        ina = np.ascontiguousarray(
            np.concatenate(
                [t16, tp0, pt16, scal16, zeros16, nbeta16, btmid16,
                 shift16, np.eye(P16, dtype=np.float32), mci8],
                axis=1, dtype=np.float32,
            )
        )
        inp = np.ascontiguousarray(
            np.concatenate(
                [np.vstack([np.concatenate([m_beta, pri8], axis=1),
                            np.zeros((8, 64 + PCOL), np.float32)]),
                 hmask],
                axis=1, dtype=np.float32,
            )
        )
        in_maps.append({"inA": ina, "inP": inp})
    return in_maps


def combine_outputs(results, event_times, mu, gamma, alpha_kernel, beta_kernel):
    t = np.asarray(event_times, dtype=np.float32)
    beta = np.asarray(beta_kernel, dtype=np.float64)
    alpha = np.asarray(alpha_kernel, dtype=np.float64)
    mu_ = np.asarray(mu, dtype=np.float64)
    gamma_ = np.asarray(gamma, dtype=np.float64)

    ll_sum = sum(
        float(r["out"][0, 2]) + float(r["out"][1, 2])
        + float(r["out"][0, 3]) + float(r["out"][1, 3]) for r in results
    )
    psum = np.zeros(C, np.float64)
    for r in results:
        o = r["out"][:, 0].astype(np.float64)
        psum += o[0:8] + o[8:16]
    elast = results[NCORES - 1]["out"][8:16, 1].astype(np.float64)

    ab = alpha / beta
    exp_term = ab * ((N - 1) - elast)
    t_diff = float(t[-1]) - float(t[0])
    t_sq_diff = float(t[-1]) ** 2 - float(t[0]) ** 2
    base_terms = t_diff * mu_ + t_sq_diff * gamma_ / (2.0 * T_WINDOW)
    integral_part = float(psum @ (exp_term + base_terms)) / N
    return np.float32(-(ll_sum - integral_part))


def kernel(probability, event_times, mu, gamma, alpha_kernel, beta_kernel):
    nc = get_nc()
    in_maps = make_in_maps(
        probability, event_times, mu, gamma, alpha_kernel, beta_kernel
    )
    res = run_bass_kernel_spmd(nc, in_maps, core_ids=list(range(NCORES))).results
    return combine_outputs(
        res, event_times, mu, gamma, alpha_kernel, beta_kernel
    )
